# revision 2
# baseline (speedup 1.0000x reference)
"""W4A4 QuaRot linear kernel v2 for 8 TRN2 NeuronCores.

Algorithm (matches reference):
  w_scales = clip(max|w|,1e-6)/7 per out-row; qw = round(w/w_scales)
  x2 = fwht(x)/sqrt(4096)
  sx = clip(max|x2|,1e-6)/7 per token; qx = round(x2/sx)
  y = (qx @ qw.T) * sx * w_scales.T + bias

Sharding: x row-sharded (ms rows/core, fed transposed); weight rows
sharded (512/core) for quantization, quantized shards AllGathered as fp8.

v2 changes vs v1:
  - H128 stage via float32r matmuls (1 cyc/row at free>=256): no bf16
    hi/lo split needed.
  - One butterfly stage (t-distance 1) absorbed into the PE stage using
    +-H128 stationaries; DVE does only 4 butterfly stages.
  - bias folded into the GEMM as a rank-1 PSUM-init matmul
    (64*rsx[m]) x (bias[n]/wsc[n]) in bf16; epilogue is one Act copy
    (scale=sx per-partition) + one DVE mul by the broadcast wsc row.
  - W quantized from the transposed load only (two passes over w_t).
"""

import sys
import numpy as np

for _p in ("/opt/trn_rl_repo",):
    if _p not in sys.path:
        sys.path.insert(0, _p)

M_FULL, K, N_FULL = 8192, 4096, 4096
NCORES = 8
NS = N_FULL // NCORES   # 512 weight rows quantized per core
KT = K // 128           # 32 kappa tiles
EPS = 1e-6
MAGIC = np.float32(1.5 * 2**23)  # add/sub forces round-to-nearest-even


def _h128_np():
    i = np.arange(128)
    a = i[:, None] & i[None, :]
    pc = np.zeros((128, 128), dtype=np.int64)
    for b in range(7):
        pc += (a >> b) & 1
    return np.where(pc % 2 == 0, 1.0, -1.0).astype(np.float32)


def build_nc(ms, reps=1):
    from concourse import bass, bacc, tile, mybir

    f32 = mybir.dt.float32
    f32r = mybir.dt.float32r
    bf16 = mybir.dt.bfloat16
    fp8 = mybir.dt.float8e4
    Alu = mybir.AluOpType
    Act = mybir.ActivationFunctionType
    Ax = mybir.AxisListType
    DR = mybir.MatmulPerfMode.DoubleRow

    mt_n = ms // 128

    nc = bacc.Bacc("TRN2", target_bir_lowering=False, debug=False)

    x_t = nc.dram_tensor("x_t", [K, ms], f32, kind="ExternalInput")
    w_t = nc.dram_tensor("w_t", [K, NS], f32, kind="ExternalInput")
    bias_s = nc.dram_tensor("bias_s", [128, 4], f32, kind="ExternalInput")
    h128_in = nc.dram_tensor("h128", [128, 128], f32, kind="ExternalInput")
    ident_in = nc.dram_tensor("ident", [128, 128], f32, kind="ExternalInput")
    out = nc.dram_tensor("out", [ms, N_FULL], f32, kind="ExternalOutput")

    with tile.TileContext(nc) as tc:
        with (
            tc.tile_pool(name="dram", bufs=1, space="DRAM") as dram,
            tc.tile_pool(name="const", bufs=1) as constp,
            tc.tile_pool(name="wrk", bufs=3) as wrk,          # 16KB slots
            tc.tile_pool(name="qx", bufs=1) as qxp,           # persistent qx fp8
            tc.tile_pool(name="qwn", bufs=2) as qwnp,         # streamed qw chunks
            tc.tile_pool(name="qws", bufs=2) as qwsp,         # quantized w quarters
            tc.tile_pool(name="wred", bufs=1) as wredp,
            tc.tile_pool(name="stage", bufs=2) as stagep,     # [128,512] f32
            tc.tile_pool(name="smA", bufs=2) as smA,
            tc.tile_pool(name="pers", bufs=1) as pers,
            tc.tile_pool(name="psF", bufs=2, space="PSUM") as psF,
            tc.tile_pool(name="psG", bufs=4, space="PSUM") as psG,
            tc.tile_pool(name="psM", bufs=2, space="PSUM") as psM,
        ):
            # ---------- constants ----------
            h128f = constp.tile([128, 128], f32)
            nc.sync.dma_start(out=h128f[:, :], in_=h128_in[:, :])
            h128p = constp.tile([128, 128], f32r)
            nc.vector.tensor_copy(h128p[:, :], h128f[:, :])
            h128n = constp.tile([128, 128], f32r)
            nc.vector.tensor_scalar(h128n[:, :], h128f[:, :], -1.0, 0.0,
                                    Alu.mult, Alu.add)
            ident = constp.tile([128, 128], f32)
            nc.sync.dma_start(out=ident[:, :], in_=ident_in[:, :])
            ones = constp.tile([1, 128], f32)
            nc.vector.memset(ones[:, :], 1.0)
            biass = constp.tile([128, 4], f32)
            nc.sync.dma_start(out=biass[:, :], in_=bias_s[:, :])

            wt_view = w_t.rearrange("(t p) n -> p t n", p=128)

            for _rep in range(reps):
                # ============ W phase ============
                # pass 1: per-row (n) absmax of w, reading w^T in quarters
                wredq = wredp.tile([128, 4, NS], f32, tag="wredq")
                for q in range(4):
                    wq = wrk.tile([128, 8, NS], f32, tag="wrk")
                    nc.scalar.dma_start(out=wq[:, :, :],
                                        in_=wt_view[:, 8 * q:8 * q + 8, :])
                    nc.vector.tensor_reduce(
                        wredq[:, q, :],
                        wq.rearrange("p t n -> p n t")[:, :, :],
                        axis=Ax.X, op=Alu.max, apply_absolute_value=True,
                    )
                wmax_pn = stagep.tile([128, NS], f32, tag="wmax_pn")
                nc.vector.tensor_reduce(
                    wmax_pn[:, :],
                    wredq.rearrange("p q n -> p n q")[:, :, :],
                    axis=Ax.X, op=Alu.max,
                )
                # partition-reduce: 4 transposes then free-reduce
                wmax4 = pers.tile([128, 4], f32, tag="wmax4")
                for j in range(4):
                    ps_t = psM.tile([128, 128], f32, tag="ps_m")
                    nc.tensor.transpose(
                        ps_t[:, :], wmax_pn[:, 128 * j:128 * (j + 1)], ident[:, :]
                    )
                    accTw = smA.tile([128, 128], f32, tag="accTw")
                    nc.scalar.activation(accTw[:, :], ps_t[:, :], Act.Copy)
                    nc.vector.tensor_reduce(
                        wmax4[:, j:j + 1], accTw[:, :], axis=Ax.X, op=Alu.max,
                    )
                wsc4 = pers.tile([128, 4], f32, tag="wsc4")
                nc.vector.tensor_scalar(
                    wsc4[:, :], wmax4[:, :], float(EPS), 1.0 / 7.0,
                    Alu.max, Alu.mult,
                )
                rwsc4 = pers.tile([128, 4], f32, tag="rwsc4")
                nc.vector.reciprocal(rwsc4[:, :], wsc4[:, :])
                # gather payload: [p, tau, (wsc, bw)] with bw = bias/wsc
                gth_sb = pers.tile([128, 4, 2], f32, tag="gth_sb")
                nc.vector.tensor_copy(gth_sb[:, :, 0], wsc4[:, :])
                nc.vector.tensor_mul(gth_sb[:, :, 1], biass[:, :], rwsc4[:, :])
                gth_shard = dram.tile([NS, 2], f32)
                nc.sync.dma_start(
                    out=gth_shard.rearrange("(t p) c -> p t c", p=128),
                    in_=gth_sb[:, :, :],
                )
                gth_full = dram.tile([N_FULL, 2], f32, addr_space="Shared")
                nc.gpsimd.collective_compute(
                    "AllGather", Alu.bypass,
                    replica_groups=[list(range(NCORES))],
                    ins=[gth_shard.opt()],
                    outs=[gth_full.opt()],
                )
                # rwsc broadcast row [128, NS]
                rwscT = smA.tile([1, NS], f32, tag="rwscT")
                for j in range(4):
                    ps_w = psM.tile([128, 128], f32, tag="ps_m")
                    nc.tensor.transpose(ps_w[0:1, :], rwsc4[:, j:j + 1],
                                        ident[:, :])
                    nc.scalar.activation(
                        rwscT[:, 128 * j:128 * (j + 1)], ps_w[0:1, :], Act.Copy
                    )
                ps_b = psF.tile([128, NS], f32, tag="ps_f")
                nc.tensor.matmul(ps_b[:, :], ones[:, :], rwscT[:, :],
                                 start=True, stop=True)
                rwscb = stagep.tile([128, NS], f32, tag="rwscb")
                nc.scalar.activation(rwscb[:, :], ps_b[:, :], Act.Copy)
                # pass 2: quantize quarters, ship per-quarter to DRAM
                qwT_shard = dram.tile([K, NS], fp8)
                qsh_view = qwT_shard.rearrange("(t p) n -> p t n", p=128)
                for q in range(4):
                    wq = wrk.tile([128, 8, NS], f32, tag="wrk")
                    nc.scalar.dma_start(out=wq[:, :, :],
                                        in_=wt_view[:, 8 * q:8 * q + 8, :])
                    nc.vector.tensor_mul(
                        wq[:, :, :], wq[:, :, :],
                        rwscb[:, None, :].broadcast_to([128, 8, NS]),
                    )
                    qwq = qwsp.tile([128, 8, NS], fp8, tag="qwq")
                    nc.vector.tensor_scalar(
                        qwq[:, :, :], wq[:, :, :], float(MAGIC), float(MAGIC),
                        Alu.add, Alu.subtract,
                    )
                    nc.scalar.dma_start(
                        out=qsh_view[:, 8 * q:8 * q + 8, :], in_=qwq[:, :, :]
                    )
                qwT_full = dram.tile([NCORES * K, NS], fp8, addr_space="Shared")
                nc.gpsimd.collective_compute(
                    "AllGather", Alu.bypass,
                    replica_groups=[list(range(NCORES))],
                    ins=[qwT_shard.opt()],
                    outs=[qwT_full.opt()],
                )

                # ============ X phase ============
                qxTs = [qxp.tile([128, KT, 128], fp8, tag=f"qxT{i}", name=f"qxT{i}")
                        for i in range(mt_n)]
                sxs = [pers.tile([128, 1], f32, tag=f"sx{i}", name=f"sx{i}")
                       for i in range(mt_n)]
                rsu8s = [pers.tile([1, 128], bf16, tag=f"rsu8{i}", name=f"rsu8{i}")
                         for i in range(mt_n)]
                x_view = x_t.rearrange("(t p) m -> p t m", p=128)
                for c in range(mt_n):
                    cur = wrk.tile([128, KT, 128], f32, tag="wrk")
                    nc.sync.dma_start(out=cur[:, :, :],
                                      in_=x_view[:, :, c * 128:(c + 1) * 128])
                    # H32 on t: DVE does distances 2,4,8,16 (4 stages)
                    for s in range(1, 5):
                        d = 1 << s
                        nxt = wrk.tile([128, KT, 128],
                                       f32r if s == 4 else f32, tag="wrk")
                        a_in = cur.rearrange("p (g two d) m -> p g two d m",
                                             two=2, d=d)
                        a_out = nxt.rearrange("p (g two d) m -> p g two d m",
                                              two=2, d=d)
                        nc.vector.tensor_add(
                            a_out[:, :, 0, :, :], a_in[:, :, 0, :, :],
                            a_in[:, :, 1, :, :],
                        )
                        nc.vector.tensor_sub(
                            a_out[:, :, 1, :, :], a_in[:, :, 0, :, :],
                            a_in[:, :, 1, :, :],
                        )
                        cur = nxt
                    # PE stage: (H2 on t-distance 1) x H128 on p, fp32r.
                    # psf cols 0:256 = H128 @ (t0+t1, t2+t3),
                    #      cols 256:512 = H128 @ (t0-t1, t2-t3).
                    x2 = wrk.tile([128, KT, 128], f32, tag="wrk")
                    for g in range(KT // 4):
                        psf = psF.tile([128, 512], f32, tag="ps_f")
                        # (rep, pair, m) view -> columns (t0, t2, t0, t2)
                        ev4 = cur[:, None, 4 * g:4 * g + 4:2, :].broadcast_to(
                            [128, 2, 2, 128])
                        ov = cur[:, 4 * g + 1:4 * g + 4:2, :]
                        hp = h128p[:, :]
                        hn = h128n[:, :]
                        nc.tensor.matmul(psf[:, :], hp, ev4,
                                         start=True, stop=False,
                                         skip_group_check=True)
                        nc.tensor.matmul(psf[:, 0:256], hp, ov,
                                         start=False, stop=True,
                                         skip_group_check=True)
                        nc.tensor.matmul(psf[:, 256:512], hn, ov,
                                         start=False, stop=True,
                                         skip_group_check=True)
                        # copy back reordering (sum0, sum1, diff0, diff1)
                        # -> t indices (4g+0, 4g+2, 4g+1, 4g+3)
                        x2v = x2[:, 4 * g:4 * g + 4, :].rearrange(
                            "p (j d) m -> p d j m", d=2, j=2
                        )
                        nc.scalar.activation(
                            x2v[:, :, :, :],
                            psf.rearrange("p (d j m) -> p d j m", d=2, j=2)[
                                :, :, :, :],
                            Act.Copy,
                        )
                    # token absmax over t then over p
                    accm = smA.tile([128, 128], f32, tag="accm")
                    nc.vector.tensor_reduce(
                        accm[:, :], x2.rearrange("p t m -> p m t")[:, :, :],
                        axis=Ax.X, op=Alu.max, apply_absolute_value=True,
                    )
                    ps_a = psM.tile([128, 128], f32, tag="ps_m")
                    nc.tensor.transpose(ps_a[:, :], accm[:, :], ident[:, :])
                    accT = smA.tile([128, 128], f32, tag="accT")
                    nc.scalar.activation(accT[:, :], ps_a[:, :], Act.Copy)
                    mx = smA.tile([128, 1], f32, tag="mx")
                    nc.vector.tensor_reduce(
                        mx[:, :], accT[:, :], axis=Ax.X, op=Alu.max,
                    )
                    # sxu = max(mx, 64e-6)/7 (unnormalized); sx = sxu/64
                    sxu = smA.tile([128, 1], f32, tag="sxu")
                    nc.vector.tensor_scalar(
                        sxu[:, :], mx[:, :], float(EPS * 64.0), 1.0 / 7.0,
                        Alu.max, Alu.mult,
                    )
                    nc.vector.tensor_scalar(
                        sxs[c][:, :], sxu[:, :], 1.0 / 64.0, 0.0,
                        Alu.mult, Alu.add,
                    )
                    rsu = smA.tile([128, 1], f32, tag="rsu")
                    nc.vector.reciprocal(rsu[:, :], sxu[:, :])
                    ps_r = psM.tile([128, 128], f32, tag="ps_m")
                    nc.tensor.transpose(ps_r[0:1, :], rsu[:, :], ident[:, :])
                    rsuT = smA.tile([1, 128], f32, tag="rsuT")
                    nc.scalar.activation(rsuT[:, :], ps_r[0:1, :], Act.Copy)
                    # bf16 64*rsu row for the bias PSUM-init matmul
                    nc.scalar.activation(rsu8s[c][:, :], ps_r[0:1, :], Act.Copy,
                                         scale=64.0)
                    ps_rb = psM.tile([128, 128], f32, tag="ps_m")
                    nc.tensor.matmul(ps_rb[:, :], ones[:, :], rsuT[:, :],
                                     start=True, stop=True)
                    rsub = smA.tile([128, 128], f32, tag="rsub")
                    nc.scalar.activation(rsub[:, :], ps_rb[:, :], Act.Copy)
                    # quantize: qx = round(x2 * rsu[m]) as fp8 (in-place mul)
                    nc.vector.tensor_mul(
                        x2[:, :, :], x2[:, :, :],
                        rsub[:, None, :].broadcast_to([128, KT, 128]),
                    )
                    nc.vector.tensor_scalar(
                        qxTs[c][:, :, :], x2[:, :, :], float(MAGIC), float(MAGIC),
                        Alu.add, Alu.subtract,
                    )

                # ============ GEMM phase ============
                qwf_view = qwT_full.rearrange("(c t p) n -> c p t n",
                                              c=NCORES, p=128)
                gv = gth_full.rearrange("n c -> c n")
                for nch in range(NCORES):
                    nsl = slice(nch * NS, (nch + 1) * NS)
                    qwn = qwnp.tile([128, KT, NS], fp8, tag="qwn")
                    nc.scalar.dma_start(out=qwn[:, :, :],
                                        in_=qwf_view[nch, :, :, :])
                    wscT = smA.tile([1, NS], f32, tag="wscT")
                    nc.sync.dma_start(out=wscT[:, :], in_=gv[0:1, nsl])
                    ps_wb = psF.tile([128, 512], f32, tag="ps_f")
                    nc.tensor.matmul(ps_wb[:, :], ones[:, :], wscT[:, :],
                                     start=True, stop=True)
                    wscb = stagep.tile([128, NS], f32, tag="wscb")
                    nc.scalar.activation(wscb[:, :], ps_wb[:, :], Act.Copy)
                    bwT = smA.tile([1, NS], f32, tag="bwT")
                    nc.sync.dma_start(out=bwT[:, :], in_=gv[1:2, nsl])
                    bw8 = smA.tile([1, NS], bf16, tag="bw8")
                    nc.scalar.activation(bw8[:, :], bwT[:, :], Act.Copy)
                    for mt in range(mt_n):
                        psg = psG.tile([128, 512], f32, tag="ps_g")
                        # bias init: psg = (64 rsx)[m] x (bias/wsc)[n]
                        nc.tensor.matmul(psg[:, :], rsu8s[mt][:, :],
                                         bw8[:, :], start=True, stop=False,
                                         skip_group_check=True)
                        for kp in range(KT // 2):
                            ksl = slice(2 * kp, 2 * kp + 2)
                            nc.tensor.matmul(
                                psg[:, :],
                                qxTs[mt][:, ksl, :],
                                qwn[:, ksl, :],
                                start=False, stop=(kp == KT // 2 - 1),
                                perf_mode=DR, skip_group_check=True,
                            )
                        t1 = stagep.tile([128, NS], f32, tag="t1")
                        nc.scalar.activation(t1[:, :], psg[:, :],
                                             Act.Copy, scale=sxs[mt][:, :])
                        y = stagep.tile([128, NS], f32, tag="y")
                        nc.vector.tensor_mul(y[:, :], t1[:, :], wscb[:, :])
                        nc.sync.dma_start(
                            out=out[mt * 128:(mt + 1) * 128, nsl], in_=y[:, :]
                        )
    nc.finalize()
    return nc


_NC_CACHE = {}


def _get_nc(ms, reps=1):
    if (ms, reps) not in _NC_CACHE:
        _NC_CACHE[(ms, reps)] = build_nc(ms, reps)
    return _NC_CACHE[(ms, reps)]


def make_in_maps(x, weight, bias):
    m = x.shape[0]
    ms = m // NCORES
    h128 = _h128_np()
    ident = np.eye(128, dtype=np.float32)
    in_maps = []
    for i in range(NCORES):
        xs = x[i * ms:(i + 1) * ms, :]
        ws = weight[i * NS:(i + 1) * NS, :]
        bs = bias[i * NS:(i + 1) * NS].reshape(4, 128).T  # [p, tau]
        in_maps.append({
            "x_t": np.ascontiguousarray(xs.T),
            "w_t": np.ascontiguousarray(ws.T),
            "bias_s": np.ascontiguousarray(bs.astype(np.float32)),
            "h128": h128,
            "ident": ident,
        })
    return in_maps


def kernel(x, weight, bias):
    from concourse.bass_utils import run_bass_kernel_spmd

    x = np.asarray(x, dtype=np.float32)
    weight = np.asarray(weight, dtype=np.float32)
    bias = np.asarray(bias, dtype=np.float32)
    ms = x.shape[0] // NCORES
    nc = _get_nc(ms)
    res = run_bass_kernel_spmd(
        nc, make_in_maps(x, weight, bias), core_ids=list(range(NCORES))
    )
    outs = [np.asarray(res.results[i]["out"], dtype=np.float32) for i in range(NCORES)]
    return np.concatenate(outs, axis=0)
